# revision 1
# baseline (speedup 1.0000x reference)
"""Trainium2 Bass kernel for retrieval_knn (nn_CLI_v1_63702954934484).

Reference computation (per batch b):
    dist[n,m] = ||ca[n] - cb[m]|| / 128                         [Na, Nb]
    idx       = argtop4-smallest(dist[n,:])                     [Na, 4]
    dw        = R - clip(dist_top4, 0, R)                       [Na, 4]
    h         = [b_f, a_f - b_f]  (b_f = feats_b[idx])          [Na, 4, 2D]
    fused     = sum_k relu(h @ W + bias) * dw                   [Na, D]
    out       = [feats_a, fused]                                [Na, 2D]

Kernel restructure (v2, fp16-centric):
  * h @ W + bias = a_f @ W2 + b_f @ (W1 - W2) + bias. Precompute
    Ya = feats_a @ W2 (+bias) and Yb = feats_b @ (W1-W2) per batch, then
    gather ROWS of Yb. All feature matmuls run in fp16 (1 cyc/row on PE vs
    4 for fp32), with lhsT pre-transposed on the host (no PE transposes).
  * Distances: the matmul computes packed = -(dist2 + m/2048) in one K=8
    fp16 matmul of lifted coords. All lifted values are exactly
    representable in fp16 and all products/sums are exact in fp32
    accumulation (for the small dist2 that can enter the top-4), so the
    ordering is bit-identical to the fp32 reference; ties break by smaller
    index = jax.lax.top_k behavior. A single DVE max8 pass then yields both
    the top-4 distances AND the neighbor indices (unpacked arithmetically)
    -- no second max_index scan.
  * top-4 via max8 on two 1024-wide PSUM halves + an 8+8 -> 8 merge.
  * neighbor rows fetched with gpsimd dma_gather, two n-tiles per call
    (1024 rows, ~8.6us of Q7 descriptor-gen) vs 4 indirect DMAs per tile.
    The int16 index list must be "wrapped in 16 partitions and replicated
    across cores" (the Q7 ucode reads the 16-partition group of whichever
    physical core runs it): built per half-batch with PE transposes (fold
    queries into the groups) + a [16,128] one-hot matmul (replicate to all
    8 groups).
  * fused output is written fp16 (well within the 2e-2 gate); the host
    upcasts and concatenates the feats_a passthrough half.

Sharding: data-parallel over batch (16 batches -> 8 cores x 2).
"""

import sys

sys.path.insert(0, "/opt/trn_rl_repo")

import numpy as np

import concourse.bass as bass
import concourse.mybir as mybir
import concourse.tile as tile
from concourse import bacc
from concourse.bass import IndirectOffsetOnAxis
from concourse.library_config import mlp as mlp_lib

F32 = mybir.dt.float32
F16 = mybir.dt.float16
U32 = mybir.dt.uint32
I16 = mybir.dt.int16

P = 128          # partitions
D = 512          # feature dim
KNN = 4          # neighbors
R = 0.5
FULL_SCALE = 128.0

B = 16           # full batch
N_CORES = 8
BLOC = B // N_CORES  # batches per core

NA = 2048
NB = 2048
NT = NA // P     # n-tiles per batch
DT = D // P      # 128-chunks of the feature dim
HALF = 1024      # distance column chunk (2 PSUM banks)

AF = mybir.ActivationFunctionType
ALU = mybir.AluOpType

USE_DMA_GATHER = True   # one dma_gather/tile vs 4 indirect DMAs/tile


def build_bass(bloc=BLOC, na=NA, nb=NB, with_bias=False):
    nc = bacc.Bacc("TRN2", debug=False)
    fatT = nc.dram_tensor("fatT", [bloc, DT, P, na], F16, kind="ExternalInput").ap()
    fbtT = nc.dram_tensor("fbtT", [bloc, DT, P, nb], F16, kind="ExternalInput").ap()
    phia = nc.dram_tensor("phia", [bloc, 8, na], F16, kind="ExternalInput").ap()
    phib = nc.dram_tensor("phib", [bloc, 8, nb], F16, kind="ExternalInput").ap()
    w2 = nc.dram_tensor("w2", [DT, P, D], F16, kind="ExternalInput").ap()
    wd = nc.dram_tensor("wd", [DT, P, D], F16, kind="ExternalInput").ap()
    biasw = nc.dram_tensor("biasw", [1, D], F16, kind="ExternalInput").ap()
    ident = nc.dram_tensor("ident", [P, P], F16, kind="ExternalInput").ap()
    rep = nc.dram_tensor("rep", [16, P], F16, kind="ExternalInput").ap()
    out = nc.dram_tensor("out", [bloc, na, D], F16, kind="ExternalOutput").ap()

    if USE_DMA_GATHER:
        nc.gpsimd.load_library(mlp_lib)
    with tile.TileContext(nc) as tc:
        _kern(tc, fatT, fbtT, phia, phib, w2, wd, biasw, ident, rep, out,
              bloc=bloc, na=na, nb=nb, with_bias=with_bias)
    nc.compile()
    return nc


def _kern(tc, fatT, fbtT, phia, phib, w2, wd, biasw, ident, rep, out, *,
          bloc, na, nb, with_bias):
    nc = tc.nc
    nt = na // P
    with (
        tc.tile_pool(name="const", bufs=1) as cpool,
        tc.tile_pool(name="wpool", bufs=1) as wpool,
        tc.tile_pool(name="feat", bufs=2) as fpool,
        tc.tile_pool(name="phi", bufs=2) as phipool,
        tc.tile_pool(name="tk", bufs=2) as tkpool,
        tc.tile_pool(name="ext", bufs=2) as epool,
        tc.tile_pool(name="io", bufs=3) as iopool,
        tc.tile_pool(name="gat", bufs=3) as gpool,
        tc.tile_pool(name="mlp", bufs=2) as mpool,
        tc.tile_pool(name="dram", bufs=2, space="DRAM") as dpool,
        tc.tile_pool(name="dist_ps", bufs=2, space="PSUM") as dps,
        tc.tile_pool(name="mm_ps", bufs=3, space="PSUM") as mmps,
        tc.tile_pool(name="tp_ps", bufs=1, space="PSUM") as tpps,
    ):
        rconst = cpool.tile([P, 1], F32, name="rconst")
        nc.vector.memset(rconst, R)
        ones_t = cpool.tile([1, P], F16, name="ones_t")
        nc.vector.memset(ones_t, 1.0)

        # resident weights (fp16): w2 / wd chunks [128, j, 512]
        w2_sb = wpool.tile([P, DT, D], F16, name="w2_sb")
        wd_sb = wpool.tile([P, DT, D], F16, name="wd_sb")
        for j in range(DT):
            nc.sync.dma_start(out=w2_sb[:, j, :], in_=w2[j])
            nc.sync.dma_start(out=wd_sb[:, j, :], in_=wd[j])
        bias_sb = wpool.tile([1, D], F16, name="bias_sb")
        if with_bias:
            nc.sync.dma_start(out=bias_sb, in_=biasw)
        ident_sb = wpool.tile([P, P], F16, name="ident_sb")
        rep_sb = wpool.tile([16, P], F16, name="rep_sb")
        if USE_DMA_GATHER:
            nc.sync.dma_start(out=ident_sb, in_=ident)
            nc.sync.dma_start(out=rep_sb, in_=rep)

        for b in range(bloc):
            # ---- per-batch loads ----
            fat_sb = fpool.tile([P, DT, na], F16, tag="fat", name="fat_sb")
            fbt_sb = fpool.tile([P, DT, nb], F16, tag="fbt", name="fbt_sb")
            # load fbt/fat in candidate/query column slices so the first
            # matmul tiles only wait for the first quarter of the load
            CS = nb // 4
            for cslice in range(4):
                c0 = cslice * CS
                nc.sync.dma_start(
                    out=fbt_sb[:, :, c0:c0 + CS].rearrange("p j c -> p j c"),
                    in_=fbtT[b, :, :, c0:c0 + CS].rearrange("j p c -> p j c"))
                nc.sync.dma_start(
                    out=fat_sb[:, :, c0:c0 + CS].rearrange("p j c -> p j c"),
                    in_=fatT[b, :, :, c0:c0 + CS].rearrange("j p c -> p j c"))
            phia_sb = phipool.tile([8, na], F16, tag="phia", name="phia_sb")
            phib_sb = phipool.tile([8, nb], F16, tag="phib", name="phib_sb")
            nc.sync.dma_start(out=phia_sb, in_=phia[b])
            nc.sync.dma_start(out=phib_sb, in_=phib[b])

            yb_dram = dpool.tile([nb, D], F16, tag="ybd", name="yb_dram")
            negd = tkpool.tile([P, nt, 8], F32, tag="negd", name="negd")

            # ---- stage 0: all Yb tiles first (dense PE burst); the
            # gathers need the complete yb_dram, so finishing it early
            # unblocks stage 3 as soon as each half-batch's topk lands ----
            for i in range(nt):
                yb_ps = mmps.tile([P, D], F32, tag="mm", name="yb_ps")
                for j in range(DT):
                    nc.tensor.matmul(
                        out=yb_ps, lhsT=fbt_sb[:, j, i * P:(i + 1) * P],
                        rhs=wd_sb[:, j, :],
                        start=(j == 0), stop=(j == DT - 1))
                yb_sb = iopool.tile([P, D], F16, tag="ybsb", name="yb_sb")
                nc.scalar.copy(out=yb_sb, in_=yb_ps)
                nc.sync.dma_start(out=yb_dram[i * P:(i + 1) * P, :], in_=yb_sb)

            t_all = gpool.tile([P, nt, 32], I16, tag="t_all", name="t_all")
            dw = epool.tile([P, nt, KNN], F32, tag="dw", name="dw")
            hh = nt // 2
            for h2 in range(2):
                i0 = h2 * hh
                nsl = slice(i0, i0 + hh)
                # ---- stage 1: distances + top8 for this half-batch ----
                for i in range(i0, i0 + hh):
                    h16 = epool.tile([P, 16], F32, tag="h16", name="h16")
                    for h in range(2):
                        dist_ps = dps.tile([P, HALF], F32, tag="dist",
                                           name="dist_ps")
                        for q in range(2):
                            c0 = h * HALF + q * 512
                            nc.tensor.matmul(
                                out=dist_ps[:, q * 512:(q + 1) * 512],
                                lhsT=phia_sb[:, i * P:(i + 1) * P],
                                rhs=phib_sb[:, c0:c0 + 512],
                                start=True, stop=True)
                        nc.vector.max(out=h16[:, h * 8:(h + 1) * 8],
                                      in_=dist_ps)
                    nc.vector.max(out=negd[:, i, :], in_=h16)

                # ---- stage 2: unpack idx / dw for this half-batch ----
                # packed = -(dist2 + m/2048); y = 2048*dist2 + m (exact int
                # in the region that matters); idx = y & 2047;
                # dist_norm = sqrt((y - idx) * 2^-25); dw = relu(R - dist_norm)
                y_f = epool.tile([P, hh, KNN], F32, tag="y_f", name="y_f")
                nc.vector.tensor_scalar(
                    out=y_f, in0=negd[:, nsl, 0:KNN], scalar1=-2048.0,
                    scalar2=None, op0=ALU.mult)
                y_u = epool.tile([P, hh, KNN], U32, tag="y_u", name="y_u")
                nc.vector.tensor_copy(out=y_u, in_=y_f)
                idx_u = epool.tile([P, hh, KNN], U32, tag="idx_u", name="idx_u")
                nc.vector.tensor_scalar(
                    out=idx_u, in0=y_u, scalar1=2047, scalar2=None,
                    op0=ALU.bitwise_and)
                d2_f = epool.tile([P, hh, KNN], F32, tag="d2_f", name="d2_f")
                nc.vector.tensor_tensor(out=d2_f, in0=y_u, in1=idx_u,
                                        op=ALU.subtract)
                dist4 = epool.tile([P, hh, KNN], F32, tag="dist4", name="dist4")
                nc.scalar.activation(out=dist4, in_=d2_f, func=AF.Sqrt,
                                     scale=float(2.0 ** -25))
                nc.scalar.activation(out=dw[:, nsl, :], in_=dist4, func=AF.Relu,
                                     scale=-1.0, bias=rconst)

                if USE_DMA_GATHER:
                    # Build the gather's index list: "wrapped in 16
                    # partitions, replicated across cores": t_all[16g+c, i,
                    # 8k+pp] = idx[16pp+c, i, k] for every group g. Pure
                    # engine ops: PE transposes fold queries into the
                    # 16-partition groups, a [16,128] one-hot matmul
                    # replicates to all 8 groups.
                    nq = hh * KNN
                    idxf = epool.tile([P, nq], F16, tag="idxf", name="idxf")
                    nc.vector.tensor_copy(
                        out=idxf, in_=idx_u[:].rearrange("p i k -> p (i k)"))
                    tpx0 = tpps.tile([nt * KNN, P], F16, tag="tpx",
                                     name="idxT_ps")
                    idxT_ps = tpx0[0:nq, :]
                    nc.tensor.transpose(out=idxT_ps, in_=idxf,
                                        identity=ident_sb)
                    idxT_sb = epool.tile([nq, P], F16, tag="idxT_sb",
                                         name="idxT_sb")
                    nc.scalar.copy(out=idxT_sb, in_=idxT_ps)
                    t16 = epool.tile([16, hh, 32], F16, tag="t16", name="t16")
                    for pp in range(8):
                        tp_big = tpps.tile([nt * KNN, P], F16, tag="tpx",
                                           name="tp_ps")
                        tp_ps = tp_big[0:16, 0:nq]
                        nc.tensor.transpose(
                            out=tp_ps, in_=idxT_sb[:, 16 * pp:16 * (pp + 1)],
                            identity=ident_sb[0:nq, 0:nq])
                        nc.scalar.copy(
                            out=t16[:, :, pp::8],
                            in_=tp_ps.rearrange("c (i k) -> c i k",
                                                i=hh, k=KNN))
                    rep_full = mmps.tile([P, D], F32, tag="mm", name="rep_ps")
                    rep_ps = rep_full[:, 0:hh * 32]
                    nc.tensor.matmul(out=rep_ps, lhsT=rep_sb,
                                     rhs=t16[:].rearrange("c i j -> c (i j)"),
                                     start=True, stop=True)
                    nc.scalar.copy(
                        out=t_all[:, nsl, :].rearrange("p i j -> p (i j)"),
                        in_=rep_ps)

                # ---- stage 3: Ya, gather, MLP combine for this half.
                # Two n-tiles per dma_gather to amortize the Q7 software
                # descriptor-generation fixed cost. ----
                GW = 2   # tiles per gather
                for ig in range(i0, i0 + hh, GW):
                    ybg = gpool.tile([P, GW, KNN, D], F16, tag="ybg",
                                     name="ybg")
                    nc.gpsimd.dma_gather(
                        out_ap=ybg[:].rearrange("p g k d -> p (g k) d"),
                        in_ap=yb_dram[:],
                        idxs_ap=t_all[:, ig:ig + GW, :].rearrange(
                            "p g j -> p (g j)"),
                        num_idxs=P * KNN * GW, num_idxs_reg=P * KNN * GW,
                        elem_size=D)
                    for i in range(ig, ig + GW):
                        ya_ps = mmps.tile([P, D], F32, tag="mm", name="ya_ps")
                        for j in range(DT):
                            nc.tensor.matmul(
                                out=ya_ps, lhsT=fat_sb[:, j, i * P:(i + 1) * P],
                                rhs=w2_sb[:, j, :],
                                start=(j == 0),
                                stop=(not with_bias and j == DT - 1))
                        if with_bias:
                            nc.tensor.matmul(out=ya_ps, lhsT=ones_t,
                                             rhs=bias_sb,
                                             start=False, stop=True)
                        ya_sb = iopool.tile([P, D], F16, tag="yasb",
                                            name="ya_sb")
                        nc.scalar.copy(out=ya_sb, in_=ya_ps)

                        ybgi = ybg[:, i - ig]
                        z = mpool.tile([P, KNN, D], F16, tag="z", name="z")
                        ya_b = ya_sb[:].unsqueeze(1).broadcast_to([P, KNN, D])
                        nc.vector.tensor_tensor(out=z, in0=ybgi, in1=ya_b,
                                                op=ALU.add)
                        r = mpool.tile([P, KNN, D], F16, tag="r", name="r")
                        for k in range(KNN):
                            nc.scalar.activation(
                                out=r[:, k, :], in_=z[:, k, :],
                                func=AF.Relu, scale=dw[:, i, k:k + 1])
                        s01 = mpool.tile([P, D], F16, tag="s01", name="s01")
                        nc.vector.tensor_add(s01, r[:, 0, :], r[:, 1, :])
                        s23 = mpool.tile([P, D], F16, tag="s23", name="s23")
                        nc.vector.tensor_add(s23, r[:, 2, :], r[:, 3, :])
                        fused = mpool.tile([P, D], F16, tag="fused",
                                           name="fused")
                        nc.vector.tensor_add(fused, s01, s23)
                        nc.sync.dma_start(out=out[b, i * P:(i + 1) * P, :],
                                          in_=fused)


# ---------------------------------------------------------------------------
# host side
# ---------------------------------------------------------------------------

def _host_inputs(feats_a, feats_b, W, bias, coords_a, coords_b):
    """Host-side prep: fp16 casts, chunk transposes, lifted packed coords."""
    d = W.shape[1]
    dt = d // P
    bsz, na_, _ = feats_a.shape
    nb_ = feats_b.shape[1]

    ca = coords_a.astype(np.int64)
    cb = coords_b.astype(np.int64)
    a2 = (ca * ca).sum(-1)                      # [B, Na] ints < 48388
    b2 = (cb * cb).sum(-1)
    hiA, loA = a2 >> 11, a2 & 2047
    hiB, loB = b2 >> 11, b2 & 2047
    m_over = (np.arange(nb_, dtype=np.float32) / 2048.0)[None, :].repeat(bsz, 0)
    # packed dot = 2a.b - |a|^2 - |b|^2 - m/2048 = -(dist2 + m/2048)
    phia8 = np.stack([ca[..., 0], ca[..., 1], ca[..., 2], hiA, loA,
                      2048 * np.ones((bsz, na_), np.int64),
                      np.ones((bsz, na_), np.int64),
                      np.ones((bsz, na_), np.int64)], axis=1).astype(np.float16)
    phib8 = np.stack([2.0 * cb[..., 0], 2.0 * cb[..., 1], 2.0 * cb[..., 2],
                      -2048 * np.ones((bsz, nb_), np.float64),
                      -np.ones((bsz, nb_), np.float64),
                      -hiB.astype(np.float64), -loB.astype(np.float64),
                      -m_over.astype(np.float64)], axis=1).astype(np.float16)

    # feats chunk-transposed: [B, dt, 128, N]
    fatT = np.ascontiguousarray(
        feats_a.reshape(bsz, na_, dt, P).transpose(0, 2, 3, 1)).astype(np.float16)
    fbtT = np.ascontiguousarray(
        feats_b.reshape(bsz, nb_, dt, P).transpose(0, 2, 3, 1)).astype(np.float16)

    w2f = W[d:]                                  # applies to a_f
    wdf = W[:d] - W[d:]                          # applies to b_f
    w2c = np.ascontiguousarray(w2f.reshape(dt, P, d)).astype(np.float16)
    wdc = np.ascontiguousarray(wdf.reshape(dt, P, d)).astype(np.float16)
    biasw = bias.reshape(1, d).astype(np.float16)
    return fatT, fbtT, phia8, phib8, w2c, wdc, biasw


def kernel(**inputs):
    feats_a = np.asarray(inputs["feats_a"], dtype=np.float32)
    feats_b = np.asarray(inputs["feats_b"], dtype=np.float32)
    W = np.asarray(inputs["W"], dtype=np.float32)
    bias = np.asarray(inputs["bias"], dtype=np.float32)
    coords_a = np.asarray(inputs["coords_a"])
    coords_b = np.asarray(inputs["coords_b"])

    fatT, fbtT, phia8, phib8, w2c, wdc, biasw = _host_inputs(
        feats_a, feats_b, W, bias, coords_a, coords_b)
    with_bias = bool(np.any(bias != 0.0))
    identm = np.eye(P, dtype=np.float16)
    repm = np.zeros((16, P), np.float16)
    for p_ in range(P):
        repm[p_ % 16, p_] = 1.0

    nc = build_bass(with_bias=with_bias)

    in_maps = []
    for c in range(N_CORES):
        s = slice(c * BLOC, (c + 1) * BLOC)
        in_maps.append({
            "fatT": np.ascontiguousarray(fatT[s]),
            "fbtT": np.ascontiguousarray(fbtT[s]),
            "phia": np.ascontiguousarray(phia8[s]),
            "phib": np.ascontiguousarray(phib8[s]),
            "w2": w2c,
            "wd": wdc,
            "biasw": biasw,
            "ident": identm,
            "rep": repm,
        })

    from concourse import bass_utils
    res = bass_utils.run_bass_kernel_spmd(nc, in_maps, core_ids=list(range(N_CORES)))
    fused = np.concatenate([r["out"] for r in res.results], axis=0)
    return np.concatenate([feats_a, fused.astype(np.float32)], axis=-1)


if __name__ == "__main__":
    nc = build_bass()
    print("built ok")

